# revision 1
# baseline (speedup 1.0000x reference)
"""Trainium2 Bass kernel for nn_MHA_9603546874182.

Causal MHA: qkv proj + rope(32) + causal attention + out proj.
B=4, T=1024, C=2048, H=32, hd=64.

Sharding: 8-way tensor parallel over heads (4 heads / core).
Each core computes qkv for its 4 heads (column-parallel), rope,
causal attention, and a row-parallel partial of the output
projection. Host sums the 8 partials (+ bias).

Device layout per core:
  phase 1 (per batch): qkv = x @ Wshard.T (token-major, psum), bias add,
      rope on q/k (DVE, token-major), PE-transpose q/k -> QT/KT [hd, T],
      v (+ones column) stays token-major for the AV matmul.
  phase 2: per head/q-block, scores^T tiles [s=128, q=512] via
      matmul(lhsT=KT, rhs=QT) (K=64), causal mask add on diagonal
      tiles, exp on ACT, AV accumulate with lhsT=[V|1] giving
      ctx^T and the softmax row-sum in one psum [65, 512].
      Normalize with reciprocal + broadcast multiply.
  phase 3: partial out = ctx^T.T @ W2shard (row-parallel), psum -> DRAM.

All matmuls run as float32r (full PE rate at N>=256).
"""

import numpy as np

B, T, C, H = 4, 1024, 2048, 32
HD = C // H          # 64
NCORES = 8
HPC = H // NCORES    # 4 heads per core
SC = HPC * HD        # 256 shard channels
NTOK = B * T         # 4096
KT16 = C // 128      # 16 k tiles
MT = NTOK // 128     # 32 token tiles
MPB = T // 128       # 8 token tiles per batch
ROT = 32
NEG = -1.0e9

_CACHE = {}


def _build_nc():
    import concourse.bass as bass
    import concourse.mybir as mybir
    import concourse.tile as tile
    from concourse import bacc
    from concourse.masks import make_identity

    f32 = mybir.dt.float32
    f32r = mybir.dt.float32r
    R = lambda ap: ap.bitcast(f32r)

    nc = bacc.Bacc("TRN2")

    xt_d = nc.dram_tensor("xt", [128, KT16, MT, 128], f32r, kind="ExternalInput")
    wq_d = nc.dram_tensor("wq", [128, KT16, 3 * SC], f32r, kind="ExternalInput")
    br_d = nc.dram_tensor("br", [128, 3 * SC], f32, kind="ExternalInput")
    c1_d = nc.dram_tensor("c1", [128, MPB, SC], f32, kind="ExternalInput")
    c2_d = nc.dram_tensor("c2", [128, MPB, SC], f32, kind="ExternalInput")
    mk_d = nc.dram_tensor("mk", [128, 1024], f32, kind="ExternalInput")
    w2_d = nc.dram_tensor("w2", [128, 2, C], f32r, kind="ExternalInput")
    out_d = nc.dram_tensor("out", [MT, 128, C], f32, kind="ExternalOutput")

    with tile.TileContext(nc) as tc:
        with (
            tc.tile_pool(name="const", bufs=1) as const,
            tc.tile_pool(name="xp", bufs=3) as xp,
            tc.tile_pool(name="qkvp", bufs=3) as qkvp,
            tc.tile_pool(name="rtp", bufs=2) as rtp,
            tc.tile_pool(name="bigp", bufs=1) as bigp,
            tc.tile_pool(name="ptp", bufs=4) as ptp,
            tc.tile_pool(name="outp", bufs=3) as outp,
            tc.tile_pool(name="rsp", bufs=2) as rsp,
            tc.tile_pool(name="ps", bufs=6, space="PSUM") as ps,
            tc.tile_pool(name="pc", bufs=2, space="PSUM") as pc,
        ):
            ident = const.tile([128, 128], f32)
            make_identity(nc, ident)
            wq = const.tile([128, KT16, 3 * SC], f32r)
            nc.sync.dma_start(wq[:], wq_d[:])
            w2 = const.tile([128, 2, C], f32r)
            nc.sync.dma_start(w2[:], w2_d[:])
            br = const.tile([128, 3 * SC], f32)
            nc.sync.dma_start(br[:], br_d[:])
            c1 = const.tile([128, MPB, SC], f32)
            nc.sync.dma_start(c1[:], c1_d[:])
            c2 = const.tile([128, MPB, SC], f32)
            nc.sync.dma_start(c2[:], c2_d[:])
            mk = const.tile([128, 1024], f32)
            nc.sync.dma_start(mk[:], mk_d[:])

            brv = br[:, 2 * SC:3 * SC].rearrange("p (h d) -> p h d", h=HPC)

            for b in range(B):
                QT = bigp.tile([128, 2, T], f32r, tag="qt")
                KTt = bigp.tile([128, 2, T], f32r, tag="kt")
                Vp = bigp.tile([128, MPB, HPC, HD + 1], f32r, tag="vp")
                ctxT = bigp.tile([128, 2, T], f32r, tag="ct")
                nc.vector.memset(Vp[:, :, :, HD:HD + 1].bitcast(f32), 1.0)

                # ---- phase 1: qkv + rope + transpose ----
                for m8 in range(MPB):
                    m = b * MPB + m8
                    xt = xp.tile([128, KT16, 128], f32r)
                    nc.sync.dma_start(xt[:], xt_d[:, :, m, :])
                    psA = ps.tile([128, 512], f32, tag="ps")
                    psB = ps.tile([128, 512], f32, tag="ps")
                    for k in range(KT16):
                        nc.tensor.matmul(
                            psA[:], xt[:, k, :], wq[:, k, 0:512],
                            start=(k == 0), stop=(k == KT16 - 1))
                        nc.tensor.matmul(
                            psB[:, 0:256], xt[:, k, :], wq[:, k, 512:768],
                            start=(k == 0), stop=(k == KT16 - 1))
                    qkv = qkvp.tile([128, 512], f32)
                    nc.vector.tensor_add(qkv[:], psA[:], br[:, 0:512])
                    # v: bias add straight into Vp (token-major)
                    nc.vector.tensor_add(
                        Vp[:, m8, :, 0:HD],
                        psB[:, 0:256].rearrange("p (h d) -> p h d", h=HPC),
                        brv)
                    # rope on q (cols 0:256) and k (cols 256:512)
                    c1v = c1[:, m8, :].rearrange("p (h d) -> p h d", h=HPC)
                    c2v = c2[:, m8, :].rearrange("p (h d) -> p h d", h=HPC)
                    for base in (0, 256):
                        sec = qkv[:, base:base + 256].rearrange(
                            "p (h d) -> p h d", h=HPC)
                        rt = rtp.tile([128, 256], f32)
                        rtv = rt.rearrange("p (h d) -> p h d", h=HPC)
                        nc.vector.tensor_mul(
                            rtv[:, :, 0:16], sec[:, :, 16:32], c2v[:, :, 0:16])
                        nc.vector.tensor_mul(
                            rtv[:, :, 16:32], sec[:, :, 0:16], c2v[:, :, 16:32])
                        nc.vector.tensor_mul(sec[:], sec[:], c1v)
                        nc.vector.tensor_add(
                            sec[:, :, 0:ROT], sec[:, :, 0:ROT], rtv[:, :, 0:ROT])
                    # transpose q/k -> QT/KT
                    for ci in range(2):
                        for base, dst in ((0, QT), (256, KTt)):
                            tp = ps.tile([128, 512], f32, tag="ps")
                            nc.tensor.transpose(
                                tp[:, 0:128],
                                qkv[:, base + ci * 128: base + (ci + 1) * 128],
                                ident)
                            nc.vector.tensor_copy(
                                dst[:, ci, m8 * 128:(m8 + 1) * 128], tp[:, 0:128])

                # ---- phase 2: attention ----
                for h in range(HPC):
                    p0 = (h % 2) * 64
                    qt_h = QT[p0:p0 + 64, h // 2, :]
                    kt_h = KTt[p0:p0 + 64, h // 2, :]
                    for qb in range(2):
                        pct = pc.tile([HD + 1, 512], f32, tag="pc")
                        nst = 4 * (qb + 1)
                        for st in range(nst):
                            stp = ps.tile([128, 512], f32, tag="ps")
                            nc.tensor.matmul(
                                stp[:], kt_h[:, st * 128:(st + 1) * 128],
                                qt_h[:, qb * 512:(qb + 1) * 512],
                                start=True, stop=True)
                            r = st - 4 * qb
                            if r >= 0:
                                w = 128 * (r + 1)
                                off = 512 - 128 * r
                                nc.vector.tensor_add(
                                    stp[:, 0:w], stp[:, 0:w], mk[:, off:off + w])
                            pt = ptp.tile([128, 512], f32r)
                            nc.scalar.activation(
                                pt[:], stp[:],
                                mybir.ActivationFunctionType.Exp)
                            nc.tensor.matmul(
                                pct[:], Vp[:, st, h, :], pt[:],
                                start=(st == 0), stop=(st == nst - 1))
                        rs = rsp.tile([1, 512], f32)
                        nc.vector.reciprocal(rs[:], pct[HD:HD + 1, :])
                        rsb = rsp.tile([HD, 512], f32, tag="rsb")
                        nc.gpsimd.partition_broadcast(rsb[:], rs[:])
                        nc.vector.tensor_mul(
                            ctxT[p0:p0 + 64, h // 2, qb * 512:(qb + 1) * 512],
                            pct[0:HD, :], rsb[:])

                # ---- phase 3: out projection partial ----
                for m8 in range(MPB):
                    m = b * MPB + m8
                    for n in range(4):
                        po = ps.tile([128, 512], f32, tag="ps")
                        for j in range(2):
                            nc.tensor.matmul(
                                po[:], ctxT[:, j, m8 * 128:(m8 + 1) * 128],
                                w2[:, j, n * 512:(n + 1) * 512],
                                start=(j == 0), stop=(j == 1))
                        ot = outp.tile([128, 512], f32)
                        nc.vector.tensor_copy(ot[:], po[:])
                        nc.sync.dma_start(out_d[m, :, n * 512:(n + 1) * 512], ot[:])

    nc.finalize()
    return nc


def _host_prep(x, rope, Wqkv_w, Wqkv_b, out_w):
    """Build per-core input maps (all fp32, partition-first layouts)."""
    xf = np.ascontiguousarray(x.reshape(NTOK, C)).astype(np.float32)
    # xt[p, k, m, t] = x[m*128+t, k*128+p]
    xt = np.ascontiguousarray(
        xf.reshape(MT, 128, KT16, 128).transpose(3, 2, 0, 1))

    # rope tables (position within a batch: t = 0..1023)
    cos = rope[:, :, 0].astype(np.float32)   # [T, 16]
    sin = rope[:, :, 1].astype(np.float32)
    C1h = np.ones((T, HD), np.float32)
    C1h[:, 0:16] = cos
    C1h[:, 16:32] = cos
    C2h = np.zeros((T, HD), np.float32)
    C2h[:, 0:16] = -sin
    C2h[:, 16:32] = sin
    C1 = np.tile(C1h, (1, HPC))              # [T, 256]
    C2 = np.tile(C2h, (1, HPC))
    # c1[p, q, j] = C1[q*128+p, j]
    c1 = np.ascontiguousarray(C1.reshape(MPB, 128, SC).transpose(1, 0, 2))
    c2 = np.ascontiguousarray(C2.reshape(MPB, 128, SC).transpose(1, 0, 2))

    # causal mask table: mk[p, y] = NEG if y < p + 512 else 0
    yy = np.arange(1024)[None, :]
    pp = np.arange(128)[:, None]
    mk = np.where(yy < pp + 512, np.float32(NEG), np.float32(0.0)).astype(np.float32)

    scale = np.float32(1.0 / np.sqrt(HD))
    in_maps = []
    for g in range(NCORES):
        hs = g * SC
        Wq = Wqkv_w[hs:hs + SC, :].astype(np.float32) * scale
        Wk = Wqkv_w[C + hs:C + hs + SC, :].astype(np.float32)
        Wv = Wqkv_w[2 * C + hs:2 * C + hs + SC, :].astype(np.float32)
        Wsh = np.concatenate([Wq, Wk, Wv], axis=0)          # [768, 2048]
        # wq[p, k, j] = Wsh[j, k*128+p]
        wqa = np.ascontiguousarray(
            Wsh.T.reshape(KT16, 128, 3 * SC).transpose(1, 0, 2))
        bq = Wqkv_b[hs:hs + SC].astype(np.float32) * scale
        bk = Wqkv_b[C + hs:C + hs + SC].astype(np.float32)
        bv = Wqkv_b[2 * C + hs:2 * C + hs + SC].astype(np.float32)
        bsh = np.concatenate([bq, bk, bv])
        bra = np.ascontiguousarray(np.broadcast_to(bsh, (128, 3 * SC)))
        # w2[p, j, o] = out_w[o, g*256 + j*128 + p]
        w2a = np.ascontiguousarray(
            out_w[:, hs:hs + SC].astype(np.float32).T.reshape(
                2, 128, C).transpose(1, 0, 2))
        in_maps.append({
            "xt": xt, "wq": wqa, "br": bra, "c1": c1, "c2": c2,
            "mk": mk, "w2": w2a,
        })
    return in_maps


def kernel(x, mask, index, rope, Wqkv_w, Wqkv_b, out_w, out_b,
           k_cache, v_cache):
    from concourse.bass_utils import run_bass_kernel_spmd

    x = np.asarray(x)
    rope = np.asarray(rope)
    Wqkv_w = np.asarray(Wqkv_w)
    Wqkv_b = np.asarray(Wqkv_b)
    out_w = np.asarray(out_w)
    out_b = np.asarray(out_b)

    if "nc" not in _CACHE:
        _CACHE["nc"] = _build_nc()
    nc = _CACHE["nc"]

    in_maps = _host_prep(x, rope, Wqkv_w, Wqkv_b, out_w)
    res = run_bass_kernel_spmd(nc, in_maps, core_ids=list(range(NCORES)))

    acc = np.zeros((NTOK, C), np.float32)
    for g in range(NCORES):
        acc += res.results[g]["out"].reshape(NTOK, C)
    acc += out_b.astype(np.float32)
    return acc.reshape(B, T, C)



# revision 15
# speedup vs baseline: 1.0849x; 1.0849x over previous
"""Trainium2 Bass kernel for nn_MHA_9603546874182.

Causal MHA: qkv proj + rope(32) + causal attention + out proj.
B=4, T=1024, C=2048, H=32, hd=64.

Sharding: 8-way tensor parallel over heads (4 heads / core).
Each core computes qkv for its 4 heads (column-parallel), rope,
causal attention, and a row-parallel partial of the output
projection. Host sums the 8 bf16 partials (+ bias, incl. the v-bias
contribution folded through out_w).

All matmuls run in bf16 (1 cycle/row on the PE at 2.4 GHz vs the
2-4x slower fp32r path measured on HW). Work is spread across
engines: DVE does rope/bias/normalize, ACT does exp + the q/k
transpose-psum evacuations, Pool (gpsimd) does mask adds + psum->
sbuf copies for v and the output, PE does matmuls + transposes.
"""

import numpy as np

B, T, C, H = 4, 1024, 2048, 32
HD = C // H          # 64
NCORES = 8
HPC = H // NCORES    # 4 heads per core
SC = HPC * HD        # 256 shard channels
NTOK = B * T         # 4096
KT16 = C // 128      # 16 k tiles
MT = NTOK // 128     # 32 token tiles
MPB = T // 128       # 8 token tiles per batch
ROT = 32
NEG = -1.0e9

_CACHE = {}


def _build_nc():
    import concourse.bass as bass
    import concourse.mybir as mybir
    import concourse.tile as tile
    from concourse import bacc
    from concourse.masks import make_identity

    f32 = mybir.dt.float32
    bf16 = mybir.dt.bfloat16

    nc = bacc.Bacc("TRN2")

    xt_d = nc.dram_tensor("xt", [128, MT, KT16 * 128], bf16, kind="ExternalInput")
    wq_d = nc.dram_tensor("wq", [128, KT16, 3 * SC], bf16, kind="ExternalInput")
    br_d = nc.dram_tensor("br", [128, 2 * SC], f32, kind="ExternalInput")
    c1_d = nc.dram_tensor("c1", [128, MPB, SC], f32, kind="ExternalInput")
    c2_d = nc.dram_tensor("c2", [128, MPB, SC], f32, kind="ExternalInput")
    mk_d = nc.dram_tensor("mk", [128, 1024], bf16, kind="ExternalInput")
    w2_d = nc.dram_tensor("w2", [128, 2, C], bf16, kind="ExternalInput")
    out_d = nc.dram_tensor("out", [MT, 128, C], bf16, kind="ExternalOutput")

    with tile.TileContext(nc) as tc:
        with (
            tc.tile_pool(name="const", bufs=1) as const,
            tc.tile_pool(name="xp", bufs=3) as xp,
            tc.tile_pool(name="qkvp", bufs=3) as qkvp,
            tc.tile_pool(name="rtp", bufs=2) as rtp,
            tc.tile_pool(name="bigp", bufs=2) as bigp,
            tc.tile_pool(name="ptp", bufs=4) as ptp,
            tc.tile_pool(name="outp", bufs=3) as outp,
            tc.tile_pool(name="rsp", bufs=2) as rsp,
            tc.tile_pool(name="ps", bufs=3, space="PSUM") as ps,
            tc.tile_pool(name="st", bufs=2, space="PSUM") as stps,
            tc.tile_pool(name="tp", bufs=1, space="PSUM") as tpps,
            tc.tile_pool(name="pc", bufs=2, space="PSUM") as pc,
        ):
            ident = const.tile([128, 128], bf16)
            make_identity(nc, ident)
            wq = const.tile([128, KT16, 3 * SC], bf16)
            nc.sync.dma_start(wq[:], wq_d[:])
            w2 = const.tile([128, 2, C], bf16)
            nc.sync.dma_start(w2[:], w2_d[:])
            br = const.tile([128, 2 * SC], f32)
            nc.sync.dma_start(br[:], br_d[:])
            c1 = const.tile([128, MPB, SC], f32)
            nc.sync.dma_start(c1[:], c1_d[:])
            c2 = const.tile([128, MPB, SC], f32)
            nc.sync.dma_start(c2[:], c2_d[:])
            mk = const.tile([128, 1024], bf16)
            nc.sync.dma_start(mk[:], mk_d[:])

            for b in range(B):
                QT = bigp.tile([128, 2, T], bf16, tag="qt")
                KTt = bigp.tile([128, 2, T], bf16, tag="kt")
                Vp = bigp.tile([128, MPB, HPC, HD + 1], bf16, tag="vp")
                ctxT = bigp.tile([128, 2, T], bf16, tag="ct")
                nc.gpsimd.memset(Vp[:, :, :, HD:HD + 1], 1.0)

                # ---- phase 1: qkv + rope + transpose ----
                for m8 in range(MPB):
                    m = b * MPB + m8
                    xt = xp.tile([128, KT16, 128], bf16)
                    nc.sync.dma_start(
                        xt[:], xt_d[:, m, :].rearrange("p (k t) -> p k t", k=KT16))
                    psA = ps.tile([128, 512], f32, tag="ps")
                    psB = ps.tile([128, 512], f32, tag="ps")
                    for k in range(KT16):
                        nc.tensor.matmul(
                            psA[:], xt[:, k, :], wq[:, k, 0:512],
                            start=(k == 0), stop=(k == KT16 - 1))
                        nc.tensor.matmul(
                            psB[:, 0:256], xt[:, k, :], wq[:, k, 512:768],
                            start=(k == 0), stop=(k == KT16 - 1))
                    # v: straight copy into Vp (token-major; bias folded on host)
                    nc.vector.tensor_copy(
                        Vp[:, m8, :, 0:HD],
                        psB[:, 0:256].rearrange("p (h d) -> p h d", h=HPC))
                    # q/k: bias add + rope -> bf16 staging
                    qkv = qkvp.tile([128, 512], f32)
                    nc.vector.tensor_add(qkv[:], psA[:], br[:])
                    qk16 = qkvp.tile([128, 512], bf16, tag="qk16")
                    c1v = c1[:, m8, :].rearrange("p (h d) -> p h d", h=HPC)
                    c2v = c2[:, m8, :].rearrange("p (h d) -> p h d", h=HPC)
                    for base in (0, 256):
                        sec = qkv[:, base:base + 256].rearrange(
                            "p (h d) -> p h d", h=HPC)
                        dst = qk16[:, base:base + 256].rearrange(
                            "p (h d) -> p h d", h=HPC)
                        rt = rtp.tile([128, 256], f32)
                        rtv = rt.rearrange("p (h d) -> p h d", h=HPC)
                        nc.vector.tensor_mul(
                            rtv[:, :, 0:16], sec[:, :, 16:32], c2v[:, :, 0:16])
                        nc.vector.tensor_mul(
                            rtv[:, :, 16:32], sec[:, :, 0:16], c2v[:, :, 16:32])
                        nc.vector.tensor_mul(dst[:], sec[:], c1v)
                        nc.vector.tensor_add(
                            dst[:, :, 0:ROT], dst[:, :, 0:ROT], rtv[:, :, 0:ROT])
                    # transpose q/k -> QT/KT (bf16, 1 cyc/row)
                    tp = tpps.tile([128, 1024], bf16)
                    for si, (base, dstT) in enumerate(((0, QT), (256, KTt))):
                        for ci in range(2):
                            nc.tensor.transpose(
                                tp[:, si * 512 + ci * 128:
                                   si * 512 + (ci + 1) * 128],
                                qk16[:, base + ci * 128: base + (ci + 1) * 128],
                                ident)
                        nc.scalar.copy(
                            dstT[:, :, m8 * 128:(m8 + 1) * 128],
                            tp[:, si * 512: si * 512 + 256]
                            .rearrange("p (c t) -> p c t", c=2))

                # ---- phase 2: attention ----
                for h in range(HPC):
                    p0 = (h % 2) * 64
                    qt_h = QT[p0:p0 + 64, h // 2, :]
                    kt_h = KTt[p0:p0 + 64, h // 2, :]
                    for qb in range(2):
                        pct = pc.tile([HD + 1, 512], f32, tag="pc")
                        nst = 4 * (qb + 1)
                        for st in range(nst):
                            stp = stps.tile([128, 512], f32)
                            nc.tensor.matmul(
                                stp[:], kt_h[:, st * 128:(st + 1) * 128],
                                qt_h[:, qb * 512:(qb + 1) * 512],
                                start=True, stop=True)
                            pt = ptp.tile([128, 512], bf16)
                            nc.scalar.activation(
                                pt[:], stp[:],
                                mybir.ActivationFunctionType.Exp)
                            # causal zeroing of the diagonal tiles (0/1 bf16
                            # mask, post-exp, Pool engine: sbuf-only)
                            r = st - 4 * qb
                            if r >= 0:
                                w = 128 * (r + 1)
                                off = 512 - 128 * r
                                nc.gpsimd.tensor_mul(
                                    pt[:, 0:w], pt[:, 0:w], mk[:, off:off + w])
                            nc.tensor.matmul(
                                pct[:], Vp[:, st, h, :], pt[:],
                                start=(st == 0), stop=(st == nst - 1))
                        # evacuate denom row to sbuf (ACT honors the
                        # partition offset; the custom DVE op does not)
                        rs_s = rsp.tile([1, 512], f32, tag="rss")
                        nc.scalar.copy(rs_s[:], pct[HD:HD + 1, :])
                        rs = rsp.tile([1, 512], f32)
                        nc.vector.reciprocal_approx_fast(rs[:], rs_s[:])
                        rsb = rsp.tile([HD, 512], f32, tag="rsb")
                        nc.gpsimd.partition_broadcast(rsb[:], rs[:])
                        nc.vector.tensor_mul(
                            ctxT[p0:p0 + 64, h // 2, qb * 512:(qb + 1) * 512],
                            pct[0:HD, :], rsb[:])

                # ---- phase 3: out projection partial ----
                for m8 in range(MPB):
                    m = b * MPB + m8
                    ot = outp.tile([128, C], bf16)
                    for n in range(4):
                        po = ps.tile([128, 512], f32, tag="ps")
                        for j in range(2):
                            nc.tensor.matmul(
                                po[:], ctxT[:, j, m8 * 128:(m8 + 1) * 128],
                                w2[:, j, n * 512:(n + 1) * 512],
                                start=(j == 0), stop=(j == 1))
                        if n % 2 == 0:
                            nc.scalar.copy(ot[:, n * 512:(n + 1) * 512], po[:])
                        else:
                            nc.vector.tensor_copy(
                                ot[:, n * 512:(n + 1) * 512], po[:])
                    nc.sync.dma_start(out_d[m, :, :], ot[:])

    nc.finalize()
    return nc


def _host_prep(x, rope, Wqkv_w, Wqkv_b, out_w):
    """Build per-core input maps (partition-first layouts, bf16 matmul ins)."""
    import ml_dtypes
    bf = ml_dtypes.bfloat16

    xf = np.ascontiguousarray(x.reshape(NTOK, C)).astype(np.float32)
    # xt[p, m, k*128 + t] = x[m*128+t, k*128+p]
    xt = np.ascontiguousarray(
        xf.reshape(MT, 128, KT16, 128).transpose(3, 0, 2, 1)
        .reshape(128, MT, KT16 * 128)).astype(bf)

    # rope tables (position within a batch: t = 0..1023)
    cos = rope[:, :, 0].astype(np.float32)   # [T, 16]
    sin = rope[:, :, 1].astype(np.float32)
    C1h = np.ones((T, HD), np.float32)
    C1h[:, 0:16] = cos
    C1h[:, 16:32] = cos
    C2h = np.zeros((T, HD), np.float32)
    C2h[:, 0:16] = -sin
    C2h[:, 16:32] = sin
    C1 = np.tile(C1h, (1, HPC))              # [T, 256]
    C2 = np.tile(C2h, (1, HPC))
    # c1[p, q, j] = C1[q*128+p, j]
    c1 = np.ascontiguousarray(C1.reshape(MPB, 128, SC).transpose(1, 0, 2))
    c2 = np.ascontiguousarray(C2.reshape(MPB, 128, SC).transpose(1, 0, 2))

    # causal keep-mask table: mk[p, y] = 0 if y < p + 512 else 1
    yy = np.arange(1024)[None, :]
    pp = np.arange(128)[:, None]
    mk = np.where(yy < pp + 512, 0.0, 1.0).astype(bf)

    scale = np.float32(1.0 / np.sqrt(HD))
    in_maps = []
    for g in range(NCORES):
        hs = g * SC
        Wq = Wqkv_w[hs:hs + SC, :].astype(np.float32) * scale
        Wk = Wqkv_w[C + hs:C + hs + SC, :].astype(np.float32)
        Wv = Wqkv_w[2 * C + hs:2 * C + hs + SC, :].astype(np.float32)
        Wsh = np.concatenate([Wq, Wk, Wv], axis=0)          # [768, 2048]
        # wq[p, k, j] = Wsh[j, k*128+p]
        wqa = np.ascontiguousarray(
            Wsh.T.reshape(KT16, 128, 3 * SC).transpose(1, 0, 2)).astype(bf)
        bq = Wqkv_b[hs:hs + SC].astype(np.float32) * scale
        bk = Wqkv_b[C + hs:C + hs + SC].astype(np.float32)
        bsh = np.concatenate([bq, bk])
        bra = np.ascontiguousarray(np.broadcast_to(bsh, (128, 2 * SC)))
        # w2[p, j, o] = out_w[o, g*256 + j*128 + p]
        w2a = np.ascontiguousarray(
            out_w[:, hs:hs + SC].astype(np.float32).T.reshape(
                2, 128, C).transpose(1, 0, 2)).astype(bf)
        in_maps.append({
            "xt": xt, "wq": wqa, "br": bra, "c1": c1, "c2": c2,
            "mk": mk, "w2": w2a,
        })
    return in_maps


def kernel(x, mask, index, rope, Wqkv_w, Wqkv_b, out_w, out_b,
           k_cache, v_cache):
    from concourse.bass_utils import run_bass_kernel_spmd

    x = np.asarray(x)
    rope = np.asarray(rope)
    Wqkv_w = np.asarray(Wqkv_w)
    Wqkv_b = np.asarray(Wqkv_b)
    out_w = np.asarray(out_w)
    out_b = np.asarray(out_b)

    if "nc" not in _CACHE:
        _CACHE["nc"] = _build_nc()
    nc = _CACHE["nc"]

    in_maps = _host_prep(x, rope, Wqkv_w, Wqkv_b, out_w)
    res = run_bass_kernel_spmd(nc, in_maps, core_ids=list(range(NCORES)))

    acc = np.zeros((NTOK, C), np.float32)
    for g in range(NCORES):
        acc += res.results[g]["out"].reshape(NTOK, C).astype(np.float32)
    # out bias + v-bias folded through the output projection
    bv = Wqkv_b[2 * C:3 * C].astype(np.float32)
    acc += out_b.astype(np.float32) + bv @ out_w.astype(np.float32).T
    return acc.reshape(B, T, C)
